# revision 1
# baseline (speedup 1.0000x reference)
"""Trainium2 Bass kernel for nn_CNFBlock (CNF log-density RK4 integrator).

Contract: kernel(**inputs) takes the FULL unsharded inputs (numpy) and
returns the FULL output [16, 10000] float32.

Math (see reference):
  z0 = broadcast(emb) over SB=16; RK4 (2 steps, 4 evals) of
  dz = W2 @ softplus(Wx z + hb + t*tw) + b2,  div = c . sigmoid(pre)
  out = log_pz0 - integral(div)

Device reformulation (validated to ~1e-7 vs reference in fp32):
  * b2 folded out of the state: track y with z = y + t*b2, which turns the
    ODE bias into bias(sb, t) = hb[sb] + t * (wxt + wht + Wx@b2) -- a
    per-partition vector applied inside the Exp activation.
  * softplus via one act-table set: sp = Ln(Exp(pre+bias) + 1).
  * sigmoid folded into the divergence matmuls: sigma = 1 - Exp(-sp);
    s2 = Exp(-sp) is accumulated over the four RK4 evals sharing each
    weight (SA: w=dt/12 evals 0,3,4,7; SB: w=dt/6 evals 1,2,5,6) on the
    GpSimd engine, then out = (log_pz0 - sum(c)) + cA^T@SA + cB^T@SB.
    The constant is applied on host.
  * RK4 state update y += w_i * dz_i streams the dz PSUM twice on DVE
    (once for ytmp, once for the y accumulator) -- no extra matmuls.
  * Layout: E=128 on partitions, tokens on the free axis.
  * Sharding: core c = 4*b + q handles sb rows [8b, 8b+8) and token
    quarter [2500q, 2500(q+1)).
"""

import sys

for _p in ("/opt/trn_rl_repo", "/root/.axon_site/_ro/trn_rl_repo"):
    if _p not in sys.path:
        sys.path.append(_p)

import numpy as np

import concourse.bacc as bacc
import concourse.tile as tile
from concourse import mybir
from concourse.bass_utils import run_bass_kernel_spmd

# This kernel only uses Exp and Ln, which share one activation table set
# (natural_log_exp_and_others). The default greedy set chooser alternates
# exp_and_others <-> natural_log, inserting a ~2.7us ACT_TABLE_LOAD around
# every activation. Blank out every other set's function list (preserving
# list order, since act_func_set_id is an index into act_info.json) so the
# chooser can only pick the combined set -- one table load total.
_orig_gat = bacc.get_activation_tables


def _gat_ln_exp_only(arch):
    tables = _orig_gat(arch)
    pref = "natural_log_exp_and_others"
    if pref not in tables:
        return tables
    return {
        name: (funcs if name == pref else type(funcs)())
        for name, funcs in tables.items()
    }


bacc.get_activation_tables = _gat_ln_exp_only

N_CORES = 8
SB = 16
T = 10000
E = 128
DT = 0.5          # T_END / N_STEPS
TQ = 2500         # tokens per core (quarter)
SB_PER_CORE = 8
W = 1024          # macro width (2 PSUM banks)
SUB = 512         # matmul moving-dim tile (1 PSUM bank)

T_EVALS = [0.0, 0.25, 0.25, 0.5, 0.5, 0.75, 0.75, 1.0]
A_COEFS = [0.25, 0.25, 0.5]                    # dt/2, dt/2, dt
W_COEFS = [DT / 6.0, DT / 3.0, DT / 3.0, DT / 6.0]
SA_EVALS = (0, 3, 4, 7)                        # weight dt/12
SB_EVALS = (1, 2, 5, 6)                        # weight dt/6

_F32 = mybir.dt.float32
_F32R = mybir.dt.float32r


def _macros():
    out = []
    off = 0
    while off < TQ:
        w = min(W, TQ - off)
        out.append((off, w))
        off += w
    return out  # [1024, 1024, 452]


def _subs(w):
    out = []
    off = 0
    while off < w:
        f = min(SUB, w - off)
        out.append((off, f))
        off += f
    return out


def _dmacros():
    # wide dz tiles: [2048, 452] (4 PSUM banks + 1)
    return [(0, 2048), (2048, TQ - 2048)]


def build_module(repeat: int = 1):
    nc = bacc.Bacc("TRN2", target_bir_lowering=False, debug=False)
    add = mybir.AluOpType.add
    mult = mybir.AluOpType.mult
    Exp = mybir.ActivationFunctionType.Exp
    Ln = mybir.ActivationFunctionType.Ln

    embT = nc.dram_tensor("embT", [E, TQ], _F32R, kind="ExternalInput")
    biasT = nc.dram_tensor("biasT", [E, SB_PER_CORE * 8], _F32, kind="ExternalInput")
    wxT = nc.dram_tensor("wxT", [E, E], _F32R, kind="ExternalInput")
    w2T = nc.dram_tensor("w2T", [E, E], _F32R, kind="ExternalInput")
    cAB = nc.dram_tensor("cAB", [E, 2], _F32R, kind="ExternalInput")
    outd = nc.dram_tensor("out", [SB_PER_CORE, TQ], _F32, kind="ExternalOutput")

    with tile.TileContext(nc) as tc:
        with (
            tc.tile_pool(name="const", bufs=1) as cp,
            tc.tile_pool(name="acts", bufs=3) as wp,
            tc.tile_pool(name="accum", bufs=2) as ac,
            tc.tile_pool(name="ytmp", bufs=3) as yt,
            tc.tile_pool(name="stage", bufs=1) as sg,
            tc.tile_pool(name="ps_pre", bufs=2, space="PSUM") as pp,
            tc.tile_pool(name="ps_dzt", bufs=1, space="PSUM") as pt,
            tc.tile_pool(name="ps_div", bufs=2, space="PSUM") as pd,
        ):
            embS = cp.tile([E, TQ], _F32R)
            nc.sync.dma_start(out=embS[:], in_=embT.ap())
            biasS = cp.tile([E, SB_PER_CORE * 8], _F32)
            nc.sync.dma_start(out=biasS[:], in_=biasT.ap())
            wxS = cp.tile([E, E], _F32R)
            nc.sync.dma_start(out=wxS[:], in_=wxT.ap())
            w2S = cp.tile([E, E], _F32R)
            nc.sync.dma_start(out=w2S[:], in_=w2T.ap())
            cabS = cp.tile([E, 2], _F32R)
            nc.sync.dma_start(out=cabS[:], in_=cAB.ap())

            for _rep in range(repeat):
                def emit_evals(sbl):
                    sa = [None]
                    sb_ = [None]
                    ypp = [None, None]
                    base = embS[:]
                    ytmp = None
                    for n in range(2):
                        for i in range(4):
                            idx = n * 4 + i
                            is_sa = idx in SA_EVALS
                            accum = sa if is_sa else sb_
                            first = idx in (0, 1)
                            if first:
                                acc0 = ac.tile([E, TQ], _F32R, name="acc0",
                                               bufs=3,
                                               tag=("sa" if is_sa else "sb"))
                                accum[0] = acc0
                                s2_dst = acc0
                            else:
                                s2t = wp.tile([E, TQ], _F32R, tag="s2t",
                                              bufs=2, name="s2t")
                                s2_dst = s2t
                            need_dzt = (i < 3) or (n == 0)
                            # full-width e so sp/u/recip run as one inst each
                            e = wp.tile([E, TQ], _F32, tag="e", bufs=2)
                            for moff, mw in _macros():
                                rhs = base[:, moff:moff + mw] if i == 0 \
                                    else ytmp[:, moff:moff + mw]
                                pre = pp.tile([E, W], _F32)
                                for soff, f in _subs(mw):
                                    nc.tensor.matmul(
                                        pre[:, soff:soff + f], wxS[:],
                                        rhs[:, soff:soff + f],
                                        start=True, stop=True,
                                    )
                                nc.scalar.activation(
                                    out=e[:, moff:moff + mw], in_=pre[:, :mw],
                                    func=Exp,
                                    bias=biasS[:, sbl * 8 + idx: sbl * 8 + idx + 1],
                                    scale=1.0,
                                )
                            sp = wp.tile([E, TQ], _F32R, tag="sp", bufs=2)
                            nc.scalar.activation(
                                out=sp[:], in_=e[:], func=Ln,
                                bias=1.0, scale=1.0,
                            )
                            if need_dzt:
                                if i < 3:
                                    nytmp = yt.tile([E, TQ], _F32R,
                                                    tag="ytile", bufs=3)
                                if n == 0:
                                    yd = ac.tile([E, TQ], _F32R, name="ynew",
                                                 bufs=2, tag="y")
                                    ysrc = base if i == 0 else ypp[0][:]
                                for moff, mw in _macros():
                                    dzt = pt.tile([E, W], _F32)
                                    for soff, f in _subs(mw):
                                        nc.tensor.matmul(
                                            dzt[:, soff:soff + f], w2S[:],
                                            sp[:, moff + soff:moff + soff + f],
                                            start=True, stop=True,
                                        )
                                    if i < 3:
                                        nc.vector.scalar_tensor_tensor(
                                            out=nytmp[:, moff:moff + mw],
                                            in0=dzt[:, :mw],
                                            scalar=A_COEFS[i],
                                            in1=base[:, moff:moff + mw],
                                            op0=mult, op1=add,
                                        )
                                    if n == 0:
                                        nc.vector.scalar_tensor_tensor(
                                            out=yd[:, moff:moff + mw],
                                            in0=dzt[:, :mw],
                                            scalar=W_COEFS[i],
                                            in1=ysrc[:, moff:moff + mw],
                                            op0=mult, op1=add,
                                        )
                                if n == 0:
                                    ypp[0] = yd
                            # s2: first macro on ACT (Exp(-sp)); the rest as
                            # 1/(1+e) via GpSimd add + DVE fast reciprocal
                            nc.scalar.activation(
                                out=s2_dst[:, 0:W], in_=sp[:, 0:W], func=Exp,
                                bias=0.0, scale=-1.0,
                            )
                            u = wp.tile([E, TQ - W], _F32, tag="u", bufs=2,
                                        name="u")
                            nc.gpsimd.tensor_scalar_add(
                                u[:], e[:, W:TQ], 1.0)
                            nc.vector.reciprocal_approx_fast(
                                out=s2_dst[:, W:TQ].bitcast(_F32), in_=u[:])
                            if not first:
                                nacc = ac.tile([E, TQ], _F32R, name="nacc",
                                               bufs=3,
                                               tag=("sa" if is_sa else "sb"))
                                nc.gpsimd.tensor_add(nacc[:], accum[0][:],
                                                     s2t[:])
                                accum[0] = nacc
                            if i < 3:
                                ytmp = nytmp[:]
                        if n == 0:
                            base = ypp[0][:]
                    return sa[0], sb_[0]

                def emit_div(sbl, saf, sbf):
                    # divergence: psum = cA^T @ SA + cB^T @ SB per 512 cols
                    stage = sg.tile([1, TQ], _F32, name="stage")
                    for soff, f in _subs(TQ):
                        div_ps = pd.tile([1, SUB], _F32, name="div_ps")
                        nc.tensor.matmul(
                            div_ps[:, :f], cabS[:, 0:1],
                            saf[:, soff:soff + f], start=True, stop=False,
                        )
                        nc.tensor.matmul(
                            div_ps[:, :f], cabS[:, 1:2],
                            sbf[:, soff:soff + f], start=False, stop=True,
                        )
                        nc.vector.tensor_copy(out=stage[:, soff:soff + f],
                                              in_=div_ps[:, :f])
                    nc.sync.dma_start(out=outd.ap()[sbl:sbl + 1, :],
                                      in_=stage[:])

                # Defer each sb's divergence block until after the next sb's
                # compute is emitted, so the scheduler overlaps the chain tail
                # with the next chain instead of stalling all engines on it.
                pending = None
                for sbl in range(SB_PER_CORE):
                    finals = emit_evals(sbl)
                    if pending is not None:
                        emit_div(*pending)
                    pending = (sbl, finals[0], finals[1])
                emit_div(*pending)
    nc.compile()
    return nc


_CACHED_NC = None


def host_prep(h, emb_matrix, log_pz0, Wx, wxt, bx, Wh, wht, bh, W2, b2):
    f = np.float32
    h = np.asarray(h, f)
    emb = np.asarray(emb_matrix, f)
    Wx = np.asarray(Wx, f); wxt = np.asarray(wxt, f); bx = np.asarray(bx, f)
    Wh = np.asarray(Wh, f); wht = np.asarray(wht, f); bh = np.asarray(bh, f)
    W2 = np.asarray(W2, f); b2 = np.asarray(b2, f)

    hb = (h.reshape(SB, E) @ Wh.T + bh + bx).astype(f)          # [16, 128]
    v = (wxt + wht + Wx @ b2).astype(f)                          # [128]
    c = np.einsum("ij,ji->j", W2, Wx).astype(f)                  # [128]
    s_c = f(c.sum(dtype=f))

    embT_np = np.ascontiguousarray(emb.T)                        # [128, T]
    wxT_np = np.ascontiguousarray(Wx.T)
    w2T_np = np.ascontiguousarray(W2.T)
    cab_np = np.ascontiguousarray(
        np.stack([c * W_COEFS[0], c * W_COEFS[1]], axis=1).astype(f))  # [128, 2]
    # column 0 = (dt/12) c pairs with the r-weighted accumulator

    t_arr = np.array(T_EVALS, f)
    in_maps = []
    for core in range(N_CORES):
        b = core // 4
        q = core % 4
        cols = []
        for sbl in range(SB_PER_CORE):
            sb = 8 * b + sbl
            cols.append(hb[sb][None, :] + t_arr[:, None] * v[None, :])  # [8,128]
        biasT_np = np.ascontiguousarray(
            np.concatenate(cols, axis=0).T.astype(f))            # [128, 64]
        in_maps.append({
            "embT": np.ascontiguousarray(embT_np[:, q * TQ:(q + 1) * TQ]),
            "biasT": biasT_np,
            "wxT": wxT_np,
            "w2T": w2T_np,
            "cAB": cab_np,
        })
    return in_maps, s_c


def kernel(h, emb_matrix, log_pz0, Wx, wxt, bx, Wh, wht, bh, W2, b2):
    global _CACHED_NC
    if _CACHED_NC is None:
        _CACHED_NC = build_module(repeat=1)
    nc = _CACHED_NC

    in_maps, s_c = host_prep(h, emb_matrix, log_pz0, Wx, wxt, bx,
                             Wh, wht, bh, W2, b2)
    res = run_bass_kernel_spmd(nc, in_maps, list(range(N_CORES)))
    P = np.zeros((SB, T), np.float32)
    for core in range(N_CORES):
        b = core // 4
        q = core % 4
        P[8 * b:8 * b + 8, q * TQ:(q + 1) * TQ] = res.results[core]["out"]
    log_pz0 = np.asarray(log_pz0, np.float32).reshape(SB, T)
    return (log_pz0 - s_c + P).astype(np.float32)



# revision 25
# speedup vs baseline: 327.8281x; 327.8281x over previous
"""Trainium2 Bass kernel for nn_CNFBlock (CNF log-density RK4 integrator).

Contract: kernel(**inputs) takes the FULL unsharded inputs (numpy) and
returns the FULL output [16, 10000] float32.

Math (see reference):
  z0 = broadcast(emb) over SB=16; RK4 (2 steps, 8 evals) of
  dz = W2 @ softplus(Wx z + hb + t*tw) + b2,  div = c . sigmoid(pre)
  out = log_pz0 - integral(div)

Device reformulation (validated to ~2e-4 vs reference):
  * ODE tracked in q-space: q := Wx y + hb with z = y + t*b2. Then
    pre = q + t*v (v = wxt+wht+Wx b2, injected via the ACT bias operand)
    and q' = M softplus(pre) with M = Wx @ W2 -- ONE matmul per eval
    instead of two, and eval-0's input q0 = Wx emb + hb is shared
    across all sb rows (computed once per core).
  * softplus via one act-table set: sp = Ln(Exp(pre)+1); eval 7 needs
    no sp at all (kappa_7 unused since z1 is not an output).
  * s2 = 1 - sigmoid(pre) = 1/(1+e) fused into ONE custom DVE op
    (BITWISE_NOT reciprocal seed + 1 Newton step + accumulate), which
    also accumulates the RK4 divergence partial sums SA/SB in the same
    pass.  out = (log_pz0 - sum(c)) + (c/12)^T SA + (c/6)^T SB.
  * RK4 state updates q_tmp/q_acc are scalar_tensor_tensor on GpSimd,
    streaming the kappa PSUM chunks.
  * Layout: E=128 on partitions, tokens on the free axis; emb shipped
    in bf16 (halves input bytes; error ~1e-4).
  * Sharding: core c = 4*b + q handles sb rows [8b, 8b+8) and token
    quarter [2500q, 2500(q+1)).
"""

import sys

for _p in ("/opt/trn_rl_repo", "/root/.axon_site/_ro/trn_rl_repo"):
    if _p not in sys.path:
        sys.path.append(_p)

import numpy as np
import ml_dtypes

import concourse.bacc as bacc
import concourse.tile as tile
from concourse import mybir
from concourse import dve_ops
from concourse.dve_spec import Spec, Src0, Src1, C0, C1, C2, Bin, AluOp
from concourse.dve_spec import lower as _dve_lower
from concourse.dve_uop import DveOpSpec
from concourse.bass_utils import dve_ver_for

# ---------------------------------------------------------------- act tables
# Only Exp, Ln and Copy are used; all live in natural_log_exp_and_others.
# Blank every other set so the greedy chooser emits exactly one table load.
_orig_gat = bacc.get_activation_tables


def _gat_ln_exp_only(arch):
    tables = _orig_gat(arch)
    pref = "natural_log_exp_and_others"
    if pref not in tables:
        return tables
    return {
        name: (funcs if name == pref else type(funcs)())
        for name, funcs in tables.items()
    }


bacc.get_activation_tables = _gat_ln_exp_only

# ------------------------------------------------------- custom DVE s2 ops
# s2 = 1/(1 + e) via the BITWISE_NOT reciprocal seed + one Newton step
# (7 ALU stages, fits the 8-stage DVE pipeline); the ACC variant adds the
# running RK4 divergence accumulator in the same pass.  abs err ~1.7e-3 on
# s2 -> ~1e-3 on the output log-density (gate is 9e-2).
_S2_C0 = 1.0          # u = e + 1
_S2_C1 = -0.235       # seed scale
_S2_C2 = 2.002        # Newton constant


def _s2_ref(in0, in1, s0, s1, imm2):
    f = np.float32
    u = (in0.astype(f) + f(s0)).astype(f)
    not_u = (~u.view(np.int32)).view(f)
    y0 = (not_u * f(s1)).astype(f)
    y1 = (y0 * (f(imm2) - u * y0)).astype(f)
    return y1


def _s2acc_ref(in0, in1, s0, s1, imm2):
    return (_s2_ref(in0, None, s0, s1, imm2) + in1.astype(np.float32)).astype(
        np.float32
    )


def _register_s2_ops():
    existing = {o.name: o for o in dve_ops.OPS}
    if "S2P1_ANT" in existing:
        return existing["S2P1_ANT"], existing["S2P1ACC_ANT"]
    u = Src0 + C0
    not_u = Bin(AluOp.BITWISE_NOT, u, u)
    y0 = not_u * C1
    y1 = y0 * (C2 - u * y0)
    made = []
    for name, body, ref in (
        ("S2P1_ANT", y1, _s2_ref),
        ("S2P1ACC_ANT", y1 + Src1, _s2acc_ref),
    ):
        spec = Spec(body=body, reference=ref)
        row = dve_ops._CUSTOM_DVE_ROW_BASE + len(dve_ops.OPS)
        assert row < 0x20, "custom-DVE opcode rows exhausted"
        shas = {}
        for ver in ("v3", "v4"):
            try:
                uops = _dve_lower(spec, ver=ver)
            except Exception:
                continue
            shas[ver] = DveOpSpec(
                name=name, opcode=row, uops=uops, rd1_en=name.endswith("ACC_ANT")
            ).sha(ver)
        op = dve_ops.DveOp(name=name, spec=spec, subdim=False, uops_sha=shas)
        dve_ops.OPS.append(op)
        dve_ops._SUB_OPCODE_FOR_NAME[name] = row
        dve_ops.CUSTOM_DVE_SPECS[name] = spec
        made.append(op)
    return made[0], made[1]


_S2P1, _S2P1ACC = _register_s2_ops()

N_CORES = 8
SB = 16
T = 10000
E = 128
DT = 0.5          # T_END / N_STEPS
TQ = 2500         # tokens per core (quarter)
SB_PER_CORE = 8
KW = 1024         # kappa PSUM macro width (2 banks)
SUB = 512         # matmul moving-dim tile (1 PSUM bank)

T_EVALS = [0.0, 0.25, 0.25, 0.5, 0.5, 0.75, 0.75, 1.0]
A_COEFS = [0.25, 0.25, 0.5]                    # dt/2, dt/2, dt
W_COEFS = [DT / 6.0, DT / 3.0, DT / 3.0, DT / 6.0]
SA_EVALS = (0, 3, 4, 7)                        # divergence weight dt/6
SB_EVALS = (1, 2, 5, 6)                        # divergence weight dt/3

_F32 = mybir.dt.float32
_F32R = mybir.dt.float32r
_BF16 = mybir.dt.bfloat16


def _chunks(total, width):
    out = []
    off = 0
    while off < total:
        w = min(width, total - off)
        out.append((off, w))
        off += w
    return out


PW = 2 * TQ      # paired plane: two sb rows side by side


def build_module(repeat: int = 1):
    nc = bacc.Bacc("TRN2", target_bir_lowering=False, debug=False)
    add = mybir.AluOpType.add
    mult = mybir.AluOpType.mult
    Exp = mybir.ActivationFunctionType.Exp
    Ln = mybir.ActivationFunctionType.Ln

    embB = nc.dram_tensor("embB", [E, TQ], _BF16, kind="ExternalInput")
    wxB = nc.dram_tensor("wxB", [E, E], _BF16, kind="ExternalInput")
    mcT = nc.dram_tensor("mcT", [E, E], _BF16, kind="ExternalInput")
    mdT = nc.dram_tensor("mdT", [E, E], _BF16, kind="ExternalInput")
    maT = nc.dram_tensor("maT", [E, E], _BF16, kind="ExternalInput")
    mbT = nc.dram_tensor("mbT", [E, E], _BF16, kind="ExternalInput")
    idT = nc.dram_tensor("idT", [E, E], _BF16, kind="ExternalInput")
    hbT = nc.dram_tensor("hbT", [E, SB_PER_CORE], _F32, kind="ExternalInput")
    tvT = nc.dram_tensor("tvT", [E, 8], _F32, kind="ExternalInput")
    cAB = nc.dram_tensor("cAB", [E, 2], _BF16, kind="ExternalInput")
    outd = nc.dram_tensor("out", [SB_PER_CORE, TQ], _F32, kind="ExternalOutput")

    with tile.TileContext(nc) as tc:
        with (
            tc.tile_pool(name="const", bufs=1) as cp,
            tc.tile_pool(name="bases", bufs=4) as bp,
            tc.tile_pool(name="acts", bufs=3) as wp,
            tc.tile_pool(name="sacc", bufs=3) as ap_,
            tc.tile_pool(name="phalf", bufs=2) as php,
            tc.tile_pool(name="q0pool", bufs=1) as q0p,
            tc.tile_pool(name="stage", bufs=1) as sg,
            tc.tile_pool(name="ps_yt", bufs=3, space="PSUM") as yp,
            tc.tile_pool(name="ps_div", bufs=2, space="PSUM") as pd,
        ):
            embS = cp.tile([E, TQ], _BF16)
            nc.sync.dma_start(out=embS[:], in_=embB.ap())
            wxS = cp.tile([E, E], _BF16)
            nc.sync.dma_start(out=wxS[:], in_=wxB.ap())
            mcS = cp.tile([E, E], _BF16)
            nc.sync.dma_start(out=mcS[:], in_=mcT.ap())
            mdS = cp.tile([E, E], _BF16)
            nc.sync.dma_start(out=mdS[:], in_=mdT.ap())
            maS = cp.tile([E, E], _BF16)
            nc.sync.dma_start(out=maS[:], in_=maT.ap())
            mbS = cp.tile([E, E], _BF16)
            nc.sync.dma_start(out=mbS[:], in_=mbT.ap())
            idS = cp.tile([E, E], _BF16)
            nc.sync.dma_start(out=idS[:], in_=idT.ap())
            hbS = cp.tile([E, SB_PER_CORE], _F32)
            nc.sync.dma_start(out=hbS[:], in_=hbT.ap())
            tvS = cp.tile([E, 8], _F32)
            nc.sync.dma_start(out=tvS[:], in_=tvT.ap())
            cabS = cp.tile([E, 2], _BF16)
            nc.sync.dma_start(out=cabS[:], in_=cAB.ap())

            for _rep in range(repeat):
                # q0 = Wx @ emb, shared by every sb row's eval 0
                q0S = q0p.tile([E, TQ], _BF16, name="q0S", tag="q0")
                for moff, mw in _chunks(TQ, KW):
                    kt = yp.tile([E, KW], _F32, tag="yt")
                    for soff, f in _chunks(mw, SUB):
                        nc.tensor.matmul(
                            kt[:, soff:soff + f], wxS[:],
                            embS[:, moff + soff:moff + soff + f],
                            start=True, stop=True,
                        )
                    nc.vector.tensor_copy(
                        out=q0S[:, moff:moff + mw], in_=kt[:, :mw])

                def emit_eval(st, n, i):
                    """One RK4 eval for one sb PAIR (plane [E, PW]).

                    The eval input for idx>=1 lives in PSUM (ytmp = I@base
                    + (a*M)@sp built by the tensor engine), read directly by
                    the Exp; only q_acc needs a DVE STT."""
                    idx = n * 4 + i
                    e = wp.tile([E, PW], _BF16, tag="e", bufs=2)
                    if idx == 0:
                        for h in (0, 1):
                            nc.scalar.activation(
                                out=e[:, h * TQ:(h + 1) * TQ], in_=q0S[:],
                                func=Exp, bias=st["hb"][h], scale=1.0)
                    else:             # ytmp / p1 chunks in PSUM
                        for off, w, yt in st["ytc"]:
                            nc.scalar.activation(
                                out=e[:, off:off + w], in_=yt[:, :w],
                                func=Exp, bias=tvS[:, idx:idx + 1], scale=1.0)
                        st["ytc"] = None
                    # fused s2 + divergence accumulate
                    is_sa = idx in SA_EVALS
                    nacc = ap_.tile([E, PW], _BF16, name="sacc",
                                    tag=("sa" if is_sa else "sb"))
                    s2chunks = _chunks(PW, KW) if idx == 7 else [(0, PW)]
                    for off, w in s2chunks:
                        if st["acc"][is_sa] is None:
                            nc.vector._custom_dve(
                                _S2P1, out=nacc[:, off:off + w],
                                in0=e[:, off:off + w],
                                s0=_S2_C0, s1=_S2_C1, imm2=_S2_C2)
                        else:
                            nc.vector._custom_dve(
                                _S2P1ACC, out=nacc[:, off:off + w],
                                in0=e[:, off:off + w],
                                in1=st["acc"][is_sa][:, off:off + w],
                                s0=_S2_C0, s1=_S2_C1, imm2=_S2_C2)
                    st["acc"][is_sa] = nacc
                    if idx > 6:
                        return
                    sp = wp.tile([E, PW], _BF16, tag="sp", bufs=5)
                    for off, w in _chunks(PW, TQ):
                        nc.scalar.activation(
                            out=sp[:, off:off + w], in_=e[:, off:off + w],
                            func=Ln, bias=1.0, scale=1.0)
                    if i < 3:
                        # next eval's input, built in PSUM: I@base + (aM)@sp
                        ma = maS if i < 2 else mbS
                        ytc = []
                        for off, w in _chunks(PW, KW):
                            yt = yp.tile([E, KW], _F32, tag="yt")
                            for soff, f in _chunks(w, SUB):
                                nc.tensor.matmul(
                                    yt[:, soff:soff + f], idS[:],
                                    st["base"][:, off + soff:off + soff + f],
                                    start=True, stop=False,
                                )
                            for soff, f in _chunks(w, SUB):
                                nc.tensor.matmul(
                                    yt[:, soff:soff + f], ma[:],
                                    sp[:, off + soff:off + soff + f],
                                    start=False, stop=True,
                                )
                            ytc.append((off, w, yt))
                        st["ytc"] = ytc
                    if n == 0:
                        st["sps"].append(sp)
                        if i in (1, 3):
                            # accumulate the step update in two passes so at
                            # most two sp planes stay live per pair:
                            #   ph = I@p0 + (M/12)@sp0 + (M/6)@sp1
                            #   p1 = I@ph + (M/6)@sp2 + (M/12)@sp3
                            first = i == 1
                            wmats = [mcS, mdS] if first else [mdS, mcS]
                            dst = (php.tile([E, PW], _BF16, name="ph")
                                   if first else
                                   bp.tile([E, PW], _BF16, tag="base",
                                           name="p1"))
                            basesrc = st["base"] if first else st["ph"]
                            ytc = []
                            for off, w in _chunks(PW, KW):
                                yt = yp.tile([E, KW], _F32, tag="yt")
                                for soff, f in _chunks(w, SUB):
                                    nc.tensor.matmul(
                                        yt[:, soff:soff + f], idS[:],
                                        basesrc[:, off + soff:off + soff + f],
                                        start=True, stop=False,
                                    )
                                for k in range(2):
                                    for soff, f in _chunks(w, SUB):
                                        nc.tensor.matmul(
                                            yt[:, soff:soff + f], wmats[k][:],
                                            st["sps"][k][:, off + soff:off + soff + f],
                                            start=False, stop=(k == 1),
                                        )
                                nc.vector.tensor_copy(
                                    out=dst[:, off:off + w], in_=yt[:, :w])
                                ytc.append((off, w, yt))
                            st["sps"] = []
                            if first:
                                st["ph"] = dst
                            else:
                                st["base"] = dst
                                st["ph"] = None
                                st["ytc"] = ytc   # eval 4 reads p1 from PSUM

                def emit_div(st):
                    sbl0 = st["sbl0"]
                    saf, sbf = st["acc"][True], st["acc"][False]
                    for h in (0, 1):
                        stage = sg.tile([1, TQ], _F32, name="stage",
                                        tag="stage", bufs=1)
                        for soff, f in _chunks(TQ, SUB):
                            div_ps = pd.tile([1, SUB], _F32, name="div_ps")
                            o = h * TQ + soff
                            nc.tensor.matmul(
                                div_ps[:, :f], cabS[:, 0:1],
                                saf[:, o:o + f], start=True, stop=False,
                            )
                            nc.tensor.matmul(
                                div_ps[:, :f], cabS[:, 1:2],
                                sbf[:, o:o + f], start=False, stop=True,
                            )
                            nc.vector.tensor_copy(
                                out=stage[:, soff:soff + f],
                                in_=div_ps[:, :f])
                        nc.sync.dma_start(
                            out=outd.ap()[sbl0 + h:sbl0 + h + 1, :],
                            in_=stage[:])

                def init_pair(sbl0):
                    p0 = bp.tile([E, PW], _BF16, tag="base", name="p0")
                    hb = []
                    for h in (0, 1):
                        hb_ap = hbS[:, sbl0 + h:sbl0 + h + 1]
                        nc.gpsimd.tensor_scalar_add(
                            p0[:, h * TQ:(h + 1) * TQ], q0S[:], hb_ap)
                        hb.append(hb_ap)
                    return {"sbl0": sbl0, "hb": hb, "base": p0,
                            "sps": [], "ytc": None, "ph": None,
                            "acc": {True: None, False: None}}

                # four sb-pairs in one continuous software pipeline,
                # phases staggered so a pair's step-boundary stall and the
                # inter-pair seams are filled by sibling work (engine queues
                # are strictly in-order).
                phases = {0: 0, 1: 2, 2: 9, 3: 11}
                states = {}
                nslots = 11 + max(phases.values())
                for slot in range(nslots):
                    live = [(pidx, slot - phases[pidx]) for pidx in range(4)
                            if 0 <= slot - phases[pidx] <= 10]
                    live.sort(key=lambda pt: pt[0])
                    for pidx, t in live:
                        if t == 0:
                            states[pidx] = init_pair(2 * pidx)
                        if t < 8:
                            emit_eval(states[pidx], t // 4, t % 4)
                        elif t == 9:
                            emit_div(states[pidx])
    nc.compile()
    return nc


def host_prep(h, emb_matrix, log_pz0, Wx, wxt, bx, Wh, wht, bh, W2, b2):
    f = np.float32
    h = np.asarray(h, f)
    emb = np.asarray(emb_matrix, f)
    Wx = np.asarray(Wx, f); wxt = np.asarray(wxt, f); bx = np.asarray(bx, f)
    Wh = np.asarray(Wh, f); wht = np.asarray(wht, f); bh = np.asarray(bh, f)
    W2 = np.asarray(W2, f); b2 = np.asarray(b2, f)

    hb = (h.reshape(SB, E) @ Wh.T + bh + bx).astype(f)          # [16, 128]
    v = (wxt + wht + Wx @ b2).astype(f)                          # [128]
    c = np.einsum("ij,ji->j", W2, Wx).astype(f)                  # [128]
    s_c = f(c.sum(dtype=f))

    embT_bf = np.ascontiguousarray(emb.T).astype(ml_dtypes.bfloat16)
    wxB_np = np.ascontiguousarray(Wx.T).astype(ml_dtypes.bfloat16)
    M = (Wx @ W2).astype(f)
    mcT_np = np.ascontiguousarray((M / 12.0).T).astype(ml_dtypes.bfloat16)
    mdT_np = np.ascontiguousarray((M / 6.0).T).astype(ml_dtypes.bfloat16)
    maT_np = np.ascontiguousarray((0.25 * M).T).astype(ml_dtypes.bfloat16)
    mbT_np = np.ascontiguousarray((0.5 * M).T).astype(ml_dtypes.bfloat16)
    idT_np = np.ascontiguousarray(np.eye(E, dtype=f)).astype(ml_dtypes.bfloat16)
    cab_np = np.ascontiguousarray(
        np.stack([c * (DT / 6.0), c * (DT / 3.0)], axis=1)
    ).astype(ml_dtypes.bfloat16)
    t_arr = np.array(T_EVALS, f)
    tv_np = np.ascontiguousarray((t_arr[:, None] * v[None, :]).T.astype(f))

    in_maps = []
    for core in range(N_CORES):
        b = core // 4
        q = core % 4
        hb_np = np.ascontiguousarray(hb[8 * b:8 * b + 8].T.astype(f))
        in_maps.append({
            "embB": np.ascontiguousarray(embT_bf[:, q * TQ:(q + 1) * TQ]),
            "wxB": wxB_np,
            "mcT": mcT_np,
            "mdT": mdT_np,
            "maT": maT_np,
            "mbT": mbT_np,
            "idT": idT_np,
            "hbT": hb_np,
            "tvT": tv_np,
            "cAB": cab_np,
        })
    return in_maps, s_c


# ------------------------------------------------------------- cached runner
_CACHED = {}


def _get_runner(repeat: int = 1):
    """Build (once) the bass module and a cached jitted SPMD executor.

    Re-implements run_bass_kernel_spmd's axon path (bass2jax.run_bass_via_pjrt)
    with the jit wrapper cached across kernel() calls, so repeat calls skip
    retrace + XLA compile.
    """
    if repeat in _CACHED:
        return _CACHED[repeat]
    import jax
    from jax.sharding import Mesh, PartitionSpec
    from jax.experimental.shard_map import shard_map
    from concourse import bass2jax
    from concourse.bass2jax import _bass_exec_p, install_neuronx_cc_hook

    nc = build_module(repeat=repeat)
    install_neuronx_cc_hook()

    partition_name = (nc.partition_id_tensor.name
                      if nc.partition_id_tensor else None)
    in_names, out_names, out_avals, out_shapes = [], [], [], []
    for alloc in nc.m.functions[0].allocations:
        if not isinstance(alloc, mybir.MemoryLocationSet):
            continue
        name = alloc.memorylocations[0].name
        if alloc.kind == "ExternalInput":
            if name != partition_name:
                in_names.append(name)
        elif alloc.kind == "ExternalOutput":
            out_names.append(name)
            shape = tuple(alloc.tensor_shape)
            dtype = mybir.dt.np(alloc.dtype)
            out_avals.append(jax.core.ShapedArray(shape, dtype))
            out_shapes.append((shape, dtype))
    n_params = len(in_names)
    n_outs = len(out_avals)
    in_names_all = list(in_names) + out_names
    if partition_name is not None:
        in_names_all.append(partition_name)
    donate = tuple(range(n_params, n_params + n_outs))

    def _body(*args):
        operands = list(args)
        if partition_name is not None:
            operands.append(bass2jax.partition_id_tensor())
        return tuple(_bass_exec_p.bind(
            *operands,
            out_avals=tuple(out_avals),
            in_names=tuple(in_names_all),
            out_names=tuple(out_names),
            lowering_input_output_aliases=(),
            sim_require_finite=True,
            sim_require_nnan=True,
            nc=nc,
        ))

    devices = jax.devices()[:N_CORES]
    mesh = Mesh(np.asarray(devices), ("core",))
    fn = jax.jit(
        shard_map(
            _body, mesh=mesh,
            in_specs=(PartitionSpec("core"),) * (n_params + n_outs),
            out_specs=(PartitionSpec("core"),) * n_outs,
            check_rep=False,
        ),
        donate_argnums=donate, keep_unused=True,
    )

    def run(in_maps):
        concat_in = [
            np.concatenate([np.asarray(in_maps[c][nm])
                            for c in range(N_CORES)], axis=0)
            for nm in in_names
        ]
        zeros = [np.zeros((N_CORES * s[0], *s[1:]), d)
                 for s, d in out_shapes]
        outs = fn(*concat_in, *zeros)
        return [
            {nm: np.asarray(outs[i]).reshape(N_CORES, *out_shapes[i][0])[c]
             for i, nm in enumerate(out_names)}
            for c in range(N_CORES)
        ]

    _CACHED[repeat] = (nc, run)
    return _CACHED[repeat]


def kernel(h, emb_matrix, log_pz0, Wx, wxt, bx, Wh, wht, bh, W2, b2):
    _, run = _get_runner(repeat=1)
    in_maps, s_c = host_prep(h, emb_matrix, log_pz0, Wx, wxt, bx,
                             Wh, wht, bh, W2, b2)
    results = run(in_maps)
    P = np.zeros((SB, T), np.float32)
    for core in range(N_CORES):
        b = core // 4
        q = core % 4
        P[8 * b:8 * b + 8, q * TQ:(q + 1) * TQ] = results[core]["out"]
    log_pz0 = np.asarray(log_pz0, np.float32).reshape(SB, T)
    return (log_pz0 - s_c + P).astype(np.float32)
